# revision 43
# baseline (speedup 1.0000x reference)
"""Trainium2 Bass kernel for nn_Drifter (Euler integration of Fourier drift ODE).

reference semantics:
    t = arange(0, 2001, 20)  (T=101 points)
    drift(x) = sin(x*orders) @ sin_w + cos(x*orders) @ cos_w   (orders=0..7)
    x_{n+1} = x_n + drift(x_n) * 20
    xt[B, T] = all iterates, wrapped to (-pi, pi];  t_mesh = broadcast t.

Device algorithm (per core, batch sharded 8 ways), in 24-bit fixed-point
turns: state v24 = (x/2pi + 0.5) * 2^24 as int32.  Per step, for each
harmonic k (folded a_k sin + b_k cos = r_k sin(2pi(k u + B_k))):
    t_k = v24 * k + B'_k        tensor_scalar on Pool (exact fp32 arith)
    p_k = t_k & 0xFFFFFF        DVE bitwise AND == phase mod 1 turn
    y_k = Sin(S24 * p_k - pi)   ACT on int32 input; equals -sin(phase_k)
Batching: the 7 t_k plus the raw state live in one contiguous
[128, 8*Fq] tile, so one wide DVE AND both reduces all 7 phases and
re-wraps the state (slot 8); one wide ACT Sin covers all 7 harmonics.
Slot 8 of the ANDed tile doubles as the wrapped state for the output
affine (x = S24*p - pi, fp16) and the next state update.  Drift sum on
the PE: PSUM D = sum_k diag(-DT r_k/2pi * 2^24) @ y_k (float32r matmuls,
PSUM accumulation), then one scalar_tensor_tensor
    v24' = (D + c0') + v24_wrapped
writes the raw next state directly into slot 8 of the NEXT step's phase
tile.  The batch runs as 4 independent column-quarter chains so each
quarter's serial tail (matmuls -> stt -> phases) hides under the other
quarters' engine work.  Host upcasts fp16 -> fp32 and transposes.
"""

import math

import numpy as np

B = 1048576
T = 101
NCORES = 8
BC = B // NCORES          # 131072 elements per core
P = 128
F = BC // P               # 1024
NQ = 4                    # independent column-quarter chains
FQ = F // NQ              # 256
NH = 7                    # harmonics 1..7
NS = NH + 1               # slots in the phase tile (7 harmonics + state)
FS_ORDER = 8
DT = 20.0
TWO_PI = 2.0 * math.pi
S24 = TWO_PI / (1 << 24)
MASK = (1 << 24) - 1
_LAST_NC = None


def _build_bass(Bint, c0i):
    """in: v24 [128, 1024] i32, wts [128, 896] f32r -> out xt_tm [101, 131072] fp16."""
    import concourse.bacc as bacc
    import concourse.mybir as mybir
    import concourse.tile as tile

    F32 = mybir.dt.float32
    F32R = mybir.dt.float32r
    F16 = mybir.dt.float16
    I32 = mybir.dt.int32
    ALU = mybir.AluOpType
    ACTF = mybir.ActivationFunctionType

    nc = bacc.Bacc("TRN2", target_bir_lowering=False)
    v0_d = nc.dram_tensor("v24", [P, F], I32, kind="ExternalInput")
    w_d = nc.dram_tensor("wts", [P, NH * P], F32R, kind="ExternalInput")
    out_d = nc.dram_tensor("xt_tm", [T, BC], F16, kind="ExternalOutput")

    nsteps = T - 1
    SSL = slice(NH * FQ, NS * FQ)  # state slot of the phase tile

    with tile.TileContext(nc) as tc:
        with (
            tc.tile_pool(name="consts", bufs=1) as const_pool,
            tc.tile_pool(name="ph", bufs=3) as ph_pool,
            tc.tile_pool(name="sins", bufs=2) as sin_pool,
            tc.tile_pool(name="outs", bufs=3) as out_pool,
            tc.psum_pool(name="acc", bufs=2) as psum_pool,
        ):
            nbias = const_pool.tile([P, 1], F32)
            nc.vector.memset(nbias[:], -math.pi)
            # warm-up Sin on a dummy tile: pulls the ACT table load into the
            # input-DMA window instead of delaying the first real sin
            warm = const_pool.tile([P, 1], F32)
            nc.scalar.activation(warm[:], nbias[:], ACTF.Sin, bias=nbias[:], scale=0.0)

            dma_engines = [nc.sync, nc.sync, nc.sync, nc.sync]

            # phase tiles for step 1: state slot preloaded with v0
            t_next = []
            for q in range(NQ):
                t_b = ph_pool.tile([P, NS * FQ], I32, tag=f"t{q}")
                nc.sync.dma_start(t_b[:, SSL], v0_d[:, q * FQ : (q + 1) * FQ])
                t_next.append(t_b)
            wts = const_pool.tile([P, NH * P], F32R)
            nc.sync.dma_start(wts[:], w_d[:])

            def emit_output(q, p_src, t_idx):
                x_o = out_pool.tile([P, FQ], F16, tag=f"xo{q}")
                nc.gpsimd.tensor_scalar(
                    x_o[:], p_src[:], S24, -math.pi, op0=ALU.mult, op1=ALU.add
                )
                dst = out_d[t_idx].rearrange("(p f) -> p f", p=P)
                dma_engines[q].dma_start(dst[:, q * FQ : (q + 1) * FQ], x_o[:])

            for n in range(1, nsteps + 1):
                t_cur = list(t_next)
                for q in range(NQ):
                    t_b = t_cur[q]
                    for k in range(1, NH + 1):
                        sl = slice((k - 1) * FQ, k * FQ)
                        if n == 1 and q == 0:
                            # pipeline fill: get quarter 0 to the ACT engine
                            # as early as possible on the idle DVE
                            peng = nc.vector if k % 2 == 0 else nc.gpsimd
                        else:
                            peng = nc.vector if (k == NH and q < 2) else nc.gpsimd
                        peng.tensor_scalar(
                            t_b[:, sl], t_b[:, SSL], float(k), float(Bint[k]),
                            op0=ALU.mult, op1=ALU.add,
                        )
                ys = []
                for q in range(NQ):
                    t_b = t_cur[q]
                    if n == 1 and q == 0:
                        # pipeline fill: AND + sin in two halves so the ACT
                        # engine starts as soon as the first slots are ready
                        h = (NH // 2) * FQ
                        nc.vector.tensor_scalar(
                            t_b[:, 0:h], t_b[:, 0:h], MASK, None, op0=ALU.bitwise_and
                        )
                        y_b = sin_pool.tile([P, NH * FQ], F32R, tag=f"y{q}")
                        nc.scalar.activation(
                            y_b[:, 0:h], t_b[:, 0:h], ACTF.Sin, bias=nbias[:], scale=S24
                        )
                        nc.vector.tensor_scalar(
                            t_b[:, h:], t_b[:, h:], MASK, None, op0=ALU.bitwise_and
                        )
                        nc.scalar.activation(
                            y_b[:, h : NH * FQ], t_b[:, h : NH * FQ], ACTF.Sin,
                            bias=nbias[:], scale=S24,
                        )
                        emit_output(q, t_b[:, SSL], n - 1)
                        ys.append(y_b)
                        continue
                    nc.vector.tensor_scalar(
                        t_b[:], t_b[:], MASK, None, op0=ALU.bitwise_and
                    )
                    emit_output(q, t_b[:, SSL], n - 1)
                    y_b = sin_pool.tile([P, NH * FQ], F32R, tag=f"y{q}")
                    nc.scalar.activation(
                        y_b[:], t_b[:, 0 : NH * FQ], ACTF.Sin, bias=nbias[:], scale=S24
                    )
                    ys.append(y_b)
                for q in range(NQ):
                    d_ps = psum_pool.tile([P, FQ], F32, tag=f"d{q}")
                    for i in range(NH):
                        nc.tensor.matmul(
                            d_ps[:],
                            wts[:, i * P : (i + 1) * P],
                            ys[q][:, i * FQ : (i + 1) * FQ],
                            start=(i == 0),
                            stop=(i == NH - 1),
                        )
                    t_nb = ph_pool.tile([P, NS * FQ], I32, tag=f"t{q}")
                    nc.vector.scalar_tensor_tensor(
                        t_nb[:, SSL], d_ps[:], c0i, t_cur[q][:, SSL],
                        op0=ALU.add, op1=ALU.add,
                    )
                    t_next[q] = t_nb

            # final slice: wrap + emit state after the last step (affine on
            # the ACT engine, which is idle during drain)
            for q in range(NQ):
                p_f = out_pool.tile([P, FQ], I32, tag=f"pf{q}")
                nc.vector.tensor_scalar(
                    p_f[:], t_next[q][:, SSL], MASK, None, op0=ALU.bitwise_and
                )
                x_f = out_pool.tile([P, FQ], F16, tag=f"xf{q}")
                if q % 2 == 0:
                    nc.scalar.activation(
                        x_f[:], p_f[:], ACTF.Copy, bias=-math.pi, scale=S24
                    )
                else:
                    nc.gpsimd.tensor_scalar(
                        x_f[:], p_f[:], S24, -math.pi, op0=ALU.mult, op1=ALU.add
                    )
                dst = out_d[nsteps].rearrange("(p f) -> p f", p=P)
                nc.sync.dma_start(dst[:, q * FQ : (q + 1) * FQ], x_f[:])

    nc.compile()
    return nc


def kernel(x0_sample, sin_weight, cos_weight, t_sample):
    from concourse import bass_utils

    x0 = np.asarray(x0_sample, dtype=np.float32)
    a = np.asarray(sin_weight, dtype=np.float64)
    b = np.asarray(cos_weight, dtype=np.float64)

    Bint = {}
    Wk = {}
    for k in range(1, FS_ORDER):
        r = math.hypot(a[k], b[k])
        phi = math.atan2(b[k], a[k])
        # k*u + B_k = k*v + (B_k - k/2) for v = u + 0.5
        Bint[k] = float(np.round(((phi / TWO_PI - k / 2.0) % 1.0) * (1 << 24)))
        Wk[k] = -(DT * r / TWO_PI) * (1 << 24)
    c0i = float((DT * b[0] / TWO_PI) * (1 << 24))

    nc = _build_bass(Bint, c0i)
    global _LAST_NC
    _LAST_NC = nc

    u0 = x0.astype(np.float64) / TWO_PI
    v24 = np.round(((u0 + 0.5) % 1.0) * (1 << 24)).astype(np.int64)
    v24 = (v24 & ((1 << 24) - 1)).astype(np.int32)
    shards = v24.reshape(NCORES, P, F)

    wts = np.zeros((P, NH * P), dtype=np.float32)
    for k in range(1, FS_ORDER):
        i = k - 1
        wts[:, i * P : (i + 1) * P] = np.diag(np.full(P, Wk[k], dtype=np.float32))

    in_maps = [
        {"v24": np.ascontiguousarray(shards[c]), "wts": wts} for c in range(NCORES)
    ]

    res = bass_utils.run_bass_kernel_spmd(nc, in_maps, core_ids=list(range(NCORES)))

    xt = np.empty((B, T), dtype=np.float32)
    for c in range(NCORES):
        xt[c * BC : (c + 1) * BC] = res.results[c]["xt_tm"].astype(np.float32).T

    t = np.arange(0.0, 2001.0, DT, dtype=np.float32)
    t_mesh = np.broadcast_to(t[None, :], (B, T))
    return (t_mesh, xt)


if __name__ == "__main__":
    rng = np.random.default_rng(0)
    x0 = rng.standard_normal(B).astype(np.float32)
    sw = (1e-4 / 8 * rng.standard_normal(8)).astype(np.float32)
    cw = (1e-4 / 8 * rng.standard_normal(8)).astype(np.float32)
    ts = rng.integers(0, 2000, B).astype(np.int32)
    tm, xt = kernel(x0, sw, cw, ts)
    print("xt", xt.shape, xt.dtype, xt[:2, :5])
